# revision 1
# baseline (speedup 1.0000x reference)
"""Batched per-adapter LoRA matmul on 8 TRN2 NeuronCores.

Problem: x [8, 4096, 2048] f32, weight [8, 2048, 64] f32
         out[b] = x[b] @ weight[b]    -> [8, 4096, 64] f32

Sharding: one batch/adapter per NeuronCore (S-LoRA style expert/data
parallelism). Per core: [4096, 2048] @ [2048, 64].

Device kernel computes out^T = w^T @ x^T so that the moving operand
(x^T, with the contraction dim D on SBUF partitions) streams through the
PE array while the small w chunk is stationary. x is transposed on the
host (layout prep), the [64, 4096] per-core result is transposed back on
the host.

Matmul dtype modes (MODE below):
  f32    - plain fp32 (4 cyc/row on PE)
  f32r   - fp32 "replicated" single-pass (1 cyc/row, ~TF32 precision)
  bf16   - x and w rounded to bf16, single pass
  bf16w2 - x bf16, w split hi+lo bf16, 2 passes
  bf16x3 - x and w split hi+lo bf16, 3 passes (~1e-6 rel err)
"""

import numpy as np
import ml_dtypes

B, S, D, R = 8, 4096, 2048, 64
N_CORES = 8
P = 128
KO = D // P  # 16 contraction chunks of 128
SB = 512  # s-block (moving free dim / PSUM bank)
NSB = S // SB  # 8

MODE = "bf16x3"

BF16 = ml_dtypes.bfloat16


def _mode_config(mode):
    """Returns (x_inputs, w_inputs, passes, np_dtype, mm_dtype_name).

    x_inputs / w_inputs: list of DRAM parameter names.
    passes: list of (w_name, x_name) accumulated into the same PSUM bank.
    """
    if mode in ("f32", "f32r"):
        return ["xt"], ["w"], [("w", "xt")], np.float32
    if mode == "bf16":
        return ["xt"], ["w"], [("w", "xt")], BF16
    if mode == "bf16w2":
        return ["xt"], ["wh", "wl"], [("wh", "xt"), ("wl", "xt")], BF16
    if mode == "bf16x3":
        return (
            ["xh", "xl"],
            ["wh", "wl"],
            [("wh", "xh"), ("wh", "xl"), ("wl", "xh")],
            BF16,
        )
    raise ValueError(mode)


def _build_nc(mode):
    from concourse import bacc
    import concourse.mybir as mybir
    import concourse.tile as tile

    x_names, w_names, passes, np_dt = _mode_config(mode)
    if np_dt is np.float32:
        dt = mybir.dt.float32
        mm_dt = mybir.dt.float32r if mode == "f32r" else mybir.dt.float32
    else:
        dt = mybir.dt.bfloat16
        mm_dt = mybir.dt.bfloat16

    nc = bacc.Bacc(None, target_bir_lowering=False)

    x_params = {
        n: nc.declare_dram_parameter(n, [D, S], dt, isOutput=False) for n in x_names
    }
    w_params = {
        n: nc.declare_dram_parameter(n, [D, R], dt, isOutput=False) for n in w_names
    }
    out_param = nc.declare_dram_parameter(
        "outT", [R, S], mybir.dt.float32, isOutput=True
    )

    def mm_ap(t):
        return t.bitcast(mm_dt) if mm_dt != t.dtype else t

    with tile.TileContext(nc) as tc:
        with (
            tc.tile_pool(name="wpool", bufs=1) as wpool,
            tc.tile_pool(name="xpool", bufs=3) as xpool,
            tc.tile_pool(name="opool", bufs=3) as opool,
            tc.tile_pool(name="psum", bufs=2, space="PSUM") as psum_pool,
        ):
            # Whole w per input: [128, 16, 64] (d-in-chunk, chunk, r)
            w_tiles = {}
            for n in w_names:
                wt = wpool.tile([P, KO, R], dt, name=f"w_{n}")
                nc.sync.dma_start(
                    wt[:], w_params[n].rearrange("(ko p) r -> p ko r", p=P)
                )
                w_tiles[n] = wt

            for s in range(NSB):
                # x^T s-block per input: [128, 16, 512]
                x_tiles = {}
                for n in x_names:
                    xt = xpool.tile([P, KO, SB], dt, name=f"x_{n}", tag=f"x_{n}")
                    nc.sync.dma_start(
                        xt[:],
                        x_params[n][:, s * SB : (s + 1) * SB].rearrange(
                            "(ko p) f -> p ko f", p=P
                        ),
                    )
                    x_tiles[n] = xt

                psum = psum_pool.tile([R, SB], mybir.dt.float32, name="ps")
                n_mm = KO * len(passes)
                i = 0
                for ko in range(KO):
                    for (wn, xn) in passes:
                        nc.tensor.matmul(
                            psum[:],
                            lhsT=mm_ap(w_tiles[wn][:, ko, :]),
                            rhs=mm_ap(x_tiles[xn][:, ko, :]),
                            start=(i == 0),
                            stop=(i == n_mm - 1),
                        )
                        i += 1

                o_tile = opool.tile([R, SB], mybir.dt.float32, name="o")
                nc.vector.tensor_copy(out=o_tile[:], in_=psum[:])
                nc.sync.dma_start(out_param[:, s * SB : (s + 1) * SB], o_tile[:])

    nc.finalize()
    return nc


_nc_cache = {}


def _get_nc(mode):
    if mode not in _nc_cache:
        _nc_cache[mode] = _build_nc(mode)
    return _nc_cache[mode]


def _prep_inputs(x, weight, mode):
    """Host-side shard + layout prep. Returns in_maps for 8 cores."""
    x_names, w_names, passes, np_dt = _mode_config(mode)
    in_maps = []
    for b in range(B):
        xt = np.ascontiguousarray(x[b].T)  # [D, S] f32
        w = np.ascontiguousarray(weight[b])  # [D, R] f32
        m = {}
        if mode in ("f32", "f32r"):
            m["xt"] = xt
            m["w"] = w
        elif mode == "bf16":
            m["xt"] = xt.astype(BF16)
            m["w"] = w.astype(BF16)
        elif mode == "bf16w2":
            m["xt"] = xt.astype(BF16)
            wh = w.astype(BF16)
            m["wh"] = wh
            m["wl"] = (w - wh.astype(np.float32)).astype(BF16)
        elif mode == "bf16x3":
            xh = xt.astype(BF16)
            m["xh"] = xh
            m["xl"] = (xt - xh.astype(np.float32)).astype(BF16)
            wh = w.astype(BF16)
            m["wh"] = wh
            m["wl"] = (w - wh.astype(np.float32)).astype(BF16)
        in_maps.append(m)
    return in_maps


def kernel(x, weight, mode=None, trace=False, _collect=None):
    """Full inputs in, full output out. Internally: 8-way batch-parallel."""
    from concourse import bass_utils

    mode = mode or MODE
    x = np.asarray(x, dtype=np.float32)
    weight = np.asarray(weight, dtype=np.float32)
    nc = _get_nc(mode)
    in_maps = _prep_inputs(x, weight, mode)
    res = bass_utils.run_bass_kernel_spmd(
        nc, in_maps, core_ids=list(range(N_CORES)), trace=trace
    )
    if _collect is not None:
        _collect.append(res)
    out = np.empty((B, S, R), dtype=np.float32)
    for b in range(B):
        out[b] = res.results[b]["outT"].T
    return out
